# revision 15
# baseline (speedup 1.0000x reference)
"""Bass/Trainium2 kernel for CustomMultiheadAttention (RoPE self-attention).

Sharding: 8 cores = 2 (batch) x 4 (head groups of 4 heads).
Each core computes, for its (b, head-group):
  qkT = Wqk_g @ x_b.T  (+bias)       -- transposed layout, split-RoPE row perm
  RoPE rotation on q/k rows           -- DVE, with host-built cos/sin tiles
  v   = x_b @ Wv_g.T  (+bias, +ones col per head)
  P.T = exp(k_rot.T^T q_rot scaled)   -- scores transposed, no max-sub needed
  accT = [v|1].T @ P.T                -- rows 0..63 = out.T, row 64 = softmax denom
  attnT = accT[0:64] * (1/denom)
  outT_partial = Wo[:, cols_g].T^T @ attnT   (1024, S) partial of final
Host: sum the 4 head-group partials per batch, add out_proj bias, transpose.

All matmuls run as float32r (full PE rate, fp32 storage).
"""

import sys
import tempfile
from contextlib import ExitStack

import numpy as np

for _p in ("/opt/trn_rl_repo",):
    if _p not in sys.path:
        sys.path.insert(0, _p)

B, S, E = 2, 2048, 1024
H, D = 16, 64
HPC = 4  # heads per core
NCORES = 8
ROPE_BASE = 10000.0

_NC = None  # cached compiled Bass program


def _emit(tc, aps):
    import concourse.bass as bass  # noqa: F401
    from concourse import mybir

    nc = tc.nc
    f32 = mybir.dt.float32
    f32r = mybir.dt.float32r
    EXP = mybir.ActivationFunctionType.Exp

    xT, wqkT, wvT, woT, bqk, bv, cosC, sinS, vones, outT = aps

    def r(ap):
        return ap if ap.dtype == f32r else ap.bitcast(f32r)

    with ExitStack() as octx:
        # tiles that live until the end of the kernel (~73KB/partition)
        late = octx.enter_context(tc.tile_pool(name="late", bufs=1))
        wo_sb = [late.tile([128, 1024], f32r, name=f"wo{k}", tag=f"wo{k}") for k in range(2)]
        for k in range(2):
            nc.sync.dma_start(out=wo_sb[k][:], in_=woT[k * 128:(k + 1) * 128, :].bitcast(f32r))
        ones64 = late.tile([1, 64], f32, name="ones64", tag="ones64")
        nc.vector.memset(ones64[:], 1.0)
        vext_sb = [late.tile([128, HPC, 65], f32r, name=f"vext{i}", tag=f"vext{i}") for i in range(16)]
        attnT_sb = [late.tile([128, S], f32r, name=f"attnT{p}", tag=f"attnT{p}") for p in range(2)]
        rot_sb = [late.tile([128, S], f32r, name=f"rot{m}", tag=f"rot{m}") for m in range(4)]

        # --- phase 1: projections (qkT, v), x streamed in 512-col blocks ---
        with ExitStack() as pctx:
            projp = pctx.enter_context(tc.tile_pool(name="projp", bufs=1))
            wqk_sb = [projp.tile([128, 512], f32r, name=f"wqk{k}", tag=f"wqk{k}") for k in range(8)]
            wv_sb = [projp.tile([128, 256], f32r, name=f"wv{k}", tag=f"wv{k}") for k in range(8)]
            for k in range(8):
                nc.sync.dma_start(out=wqk_sb[k][:], in_=wqkT[k * 128:(k + 1) * 128, :].bitcast(f32r))
                nc.sync.dma_start(out=wv_sb[k][:], in_=wvT[k * 128:(k + 1) * 128, :].bitcast(f32r))
            cos_sb = projp.tile([128, S], f32, name="cos", tag="cos")
            sin_sb = projp.tile([128, S], f32, name="sin", tag="sin")
            nc.sync.dma_start(out=cos_sb[:], in_=cosC[:])
            nc.sync.dma_start(out=sin_sb[:], in_=sinS[:])
            bqk_sb = [projp.tile([128, 1], f32, name=f"bqk{m}", tag=f"bqk{m}") for m in range(4)]
            for m in range(4):
                nc.sync.dma_start(out=bqk_sb[m][:], in_=bqk[m])
            bv_sb = projp.tile([128, HPC, 64], f32, name="bv", tag="bv")
            nc.sync.dma_start(out=bv_sb[:], in_=bv[0:1, :].to_broadcast((128, HPC, 64)))
            qkT_sb = [projp.tile([128, S], f32, name=f"qkT{m}", tag=f"qkT{m}") for m in range(4)]

            with ExitStack() as xctx:
                xpool = xctx.enter_context(tc.tile_pool(name="xTp", bufs=2))
                qk_ps = xctx.enter_context(tc.tile_pool(name="qkps", bufs=2, space="PSUM"))
                v_ps = xctx.enter_context(tc.tile_pool(name="vps", bufs=2, space="PSUM"))
                for n in range(4):
                    ns = slice(n * 512, (n + 1) * 512)
                    xn = [xpool.tile([128, 512], f32r, name=f"x{k}", tag=f"x{k}") for k in range(8)]
                    for k in range(8):
                        nc.sync.dma_start(out=xn[k][:], in_=xT[k * 128:(k + 1) * 128, ns].bitcast(f32r))
                    # qkT[m][:, ns] = (wqkT[:, m-block]).T @ xT[:, ns] + bias
                    for m in range(4):
                        ps = qk_ps.tile([128, 512], f32)
                        for k in range(8):
                            nc.tensor.matmul(
                                ps[:],
                                r(wqk_sb[k][:, m * 128:(m + 1) * 128]),
                                r(xn[k][:]),
                                start=(k == 0),
                                stop=(k == 7),
                            )
                        nc.vector.tensor_scalar_add(qkT_sb[m][:, ns], ps[:], bqk_sb[m][:])
                    # v rows for this s-block: 4 tiles of 128 rows
                    for j in range(4):
                        i = n * 4 + j
                        ps = v_ps.tile([128, 256], f32)
                        for k in range(8):
                            nc.tensor.matmul(
                                ps[:],
                                r(xn[k][:, j * 128:(j + 1) * 128]),
                                r(wv_sb[k][:]),
                                start=(k == 0),
                                stop=(k == 7),
                            )
                        nc.sync.dma_start(
                            out=vext_sb[i][:, :, 64:65],
                            in_=vones[0:1].to_broadcast((128, HPC, 1)),
                        )
                        nc.vector.tensor_add(
                            vext_sb[i][:, :, 0:64],
                            ps.rearrange("p (h d) -> p h d", h=HPC),
                            bv_sb[:],
                        )

            # --- phase 1b: RoPE rotation (split-pair layout) ---
            with ExitStack() as rctx:
                swp = rctx.enter_context(tc.tile_pool(name="swp", bufs=2))
                tmpp = rctx.enter_context(tc.tile_pool(name="tmpp", bufs=2))
                for m in range(4):
                    sw = swp.tile([128, S], f32)
                    for blk in range(4):  # swap 32-row even/odd blocks
                        src = (blk ^ 1) * 32
                        nc.sync.dma_start(
                            out=sw[blk * 32:(blk + 1) * 32, :],
                            in_=qkT_sb[m][src:src + 32, :],
                        )
                    tmp = tmpp.tile([128, S], f32)
                    nc.vector.tensor_mul(tmp[:], sw[:], sin_sb[:])
                    nc.vector.tensor_mul(rot_sb[m][:], qkT_sb[m][:], cos_sb[:])
                    nc.vector.tensor_add(rot_sb[m][:], rot_sb[m][:], tmp[:])

        # --- phase 2: attention + out-proj ---
        with ExitStack() as actx:
            sc_ps = actx.enter_context(tc.tile_pool(name="scps", bufs=2, space="PSUM"))
            oa_ps = actx.enter_context(tc.tile_pool(name="oaps", bufs=1, space="PSUM"))
            op_ps = actx.enter_context(tc.tile_pool(name="opps", bufs=2, space="PSUM"))
            ptp = actx.enter_context(tc.tile_pool(name="ptp", bufs=3))
            lbp = actx.enter_context(tc.tile_pool(name="lbp", bufs=4))
            outp = actx.enter_context(tc.tile_pool(name="outp", bufs=3))

            for sq in range(4):
                sqs = slice(sq * 512, (sq + 1) * 512)
                for p in range(2):
                    qrot, krot = rot_sb[p], rot_sb[2 + p]
                    acc = [
                        oa_ps.tile([65, 512], f32, name=f"acc{h}", tag=f"acc{h}")
                        for h in range(2)
                    ]
                    for sk in range(16):
                        ps = sc_ps.tile([128, 1024], f32)
                        for h in range(2):
                            # scores.T tile: (sk x sq) = k_rot_slice.T @ q_rot_slice
                            nc.tensor.matmul(
                                ps[:, h * 512:(h + 1) * 512],
                                r(krot[h * 64:(h + 1) * 64, sk * 128:(sk + 1) * 128]),
                                r(qrot[h * 64:(h + 1) * 64, sqs]),
                                start=True,
                                stop=True,
                            )
                        pt = ptp.tile([128, 1024], f32r)
                        nc.scalar.activation(pt[:], ps[:], EXP)
                        for h in range(2):
                            nc.tensor.matmul(
                                acc[h][:],
                                r(vext_sb[sk][:, 2 * p + h, :]),
                                r(pt[:, h * 512:(h + 1) * 512]),
                                start=(sk == 0),
                                stop=(sk == 15),
                            )
                    for h in range(2):
                        accs = lbp.tile([65, 512], f32, name="accs", tag="accs")
                        nc.vector.tensor_copy(accs[:], acc[h][:])
                        linv = lbp.tile([1, 512], f32, name="linv", tag="linv")
                        nc.vector.reciprocal(linv[:], accs[64:65, :])
                        # broadcast linv across 64 partitions via a K=1 matmul
                        # (shares the scores-pool PSUM slots; plain f32 for exactness)
                        lb_ps = sc_ps.tile([64, 512], f32, name="lb_ps", tag="ps")
                        nc.tensor.matmul(
                            lb_ps[:], ones64[:], linv[:], start=True, stop=True
                        )
                        nc.vector.tensor_mul(
                            attnT_sb[p][h * 64:(h + 1) * 64, sqs],
                            accs[0:64, :],
                            lb_ps[:],
                        )
                # out-proj for this sq block (both pairs done)
                for m in range(8):
                    ps = op_ps.tile([128, 512], f32)
                    for k2 in range(2):
                        nc.tensor.matmul(
                            ps[:],
                            r(wo_sb[k2][:, m * 128:(m + 1) * 128]),
                            r(attnT_sb[k2][:, sqs]),
                            start=(k2 == 0),
                            stop=(k2 == 1),
                        )
                    ot = outp.tile([128, 512], f32)
                    nc.vector.tensor_copy(ot[:], ps[:])
                    nc.sync.dma_start(out=outT[m * 128:(m + 1) * 128, sqs], in_=ot[:])


def _build():
    import concourse.tile as tile
    from concourse import bacc, mybir

    f32 = mybir.dt.float32
    nc = bacc.Bacc("TRN2", target_bir_lowering=False, debug=False, num_devices=NCORES)

    def din(name, shape):
        return nc.dram_tensor(name, shape, f32, kind="ExternalInput").ap()

    aps = (
        din("xT", [E, S]),
        din("wqkT", [E, 512]),
        din("wvT", [E, 256]),
        din("woT", [256, E]),
        din("bqk", [4, 128, 1]),
        din("bv", [1, HPC, 64]),
        din("cosC", [128, S]),
        din("sinS", [128, S]),
        nc.dram_tensor("vones", [1, HPC, 1], mybir.dt.float32r, kind="ExternalInput").ap(),
        nc.dram_tensor("outT", [E, S], f32, kind="ExternalOutput").ap(),
    )
    with tile.TileContext(nc) as tc:
        _emit(tc, aps)
    nc.compile()
    return nc


def _rope_tables():
    inv_freq = (
        1.0
        / np.float32(ROPE_BASE)
        ** (np.arange(0, D, 2, dtype=np.float32) / np.float32(D))
    ).astype(np.float32)
    positions = np.arange(S, dtype=np.float32)
    angles = positions[:, None] * inv_freq[None, :]  # (S, 32)
    cosT = np.cos(angles).T.astype(np.float32)  # (32, S)
    sinT = np.sin(angles).T.astype(np.float32)
    C = np.tile(cosT, (4, 1))  # (128, S)
    Ssgn = np.tile(np.concatenate([-sinT, sinT], axis=0), (2, 1))  # (128, S)
    return np.ascontiguousarray(C), np.ascontiguousarray(Ssgn)


def _prep_in_maps(query, in_proj_weight, in_proj_bias, out_proj_weight):
    x = np.asarray(query, np.float32)
    W = np.asarray(in_proj_weight, np.float32)
    bin_ = np.asarray(in_proj_bias, np.float32)
    Wo = np.asarray(out_proj_weight, np.float32)

    perm = np.concatenate([np.arange(0, D, 2), np.arange(1, D, 2)])
    scale = np.float32(1.0 / np.sqrt(D))
    Wq, Wk, Wv = W[0:E], W[E:2 * E], W[2 * E:3 * E]
    bq, bk, bv_ = bin_[0:E], bin_[E:2 * E], bin_[2 * E:3 * E]
    C, Ssgn = _rope_tables()

    in_maps = []
    for core in range(NCORES):
        b, g = core // 4, core % 4
        hs = [4 * g + j for j in range(HPC)]
        Wqk_rows = [Wq[h * D:(h + 1) * D][perm] * scale for h in hs] + [
            Wk[h * D:(h + 1) * D][perm] for h in hs
        ]
        Wqk_g = np.concatenate(Wqk_rows, axis=0)  # (512, E)
        bqk_rows = [bq[h * D:(h + 1) * D][perm] * scale for h in hs] + [
            bk[h * D:(h + 1) * D][perm] for h in hs
        ]
        bqk_g = np.concatenate(bqk_rows).reshape(4, 128, 1)
        Wv_g = Wv[hs[0] * D:(hs[-1] + 1) * D]  # (256, E)
        bv_g = bv_[hs[0] * D:(hs[-1] + 1) * D].reshape(1, HPC, 64)
        WoT_g = Wo.T[g * 256:(g + 1) * 256, :]  # (256, E)
        in_maps.append(
            {
                "xT": np.ascontiguousarray(x[b].T),
                "wqkT": np.ascontiguousarray(Wqk_g.T),
                "wvT": np.ascontiguousarray(Wv_g.T),
                "woT": np.ascontiguousarray(WoT_g),
                "bqk": np.ascontiguousarray(bqk_g),
                "bv": np.ascontiguousarray(bv_g),
                "cosC": C,
                "sinS": Ssgn,
                "vones": np.ones((1, HPC, 1), np.float32),
            }
        )
    return in_maps


def _gather(results, out_proj_bias):
    bo = np.asarray(out_proj_bias, np.float32)
    out = np.empty((B, S, E), np.float32)
    for b in range(B):
        acc = results[b * 4]["outT"].copy()
        for g in range(1, 4):
            acc += results[b * 4 + g]["outT"]
        out[b] = acc.T + bo[None, :]
    return out


_RUNNER = None


def _get_runner():
    """Build the jitted shard_map executable once; reuse across kernel() calls.

    Mirrors concourse.bass2jax.run_bass_via_pjrt's multi-core path, but caches
    the jit so later calls skip retracing (the NEFF itself is compile-cached).
    """
    global _NC, _RUNNER
    if _RUNNER is not None:
        return _RUNNER
    if _NC is None:
        _NC = _build()
    import jax
    from jax.experimental.shard_map import shard_map
    from jax.sharding import Mesh, PartitionSpec
    from concourse import bass2jax, mybir

    bass2jax.install_neuronx_cc_hook()
    nc = _NC
    assert not nc.dbg_callbacks if nc.dbg_addr is not None else True
    partition_name = (
        nc.partition_id_tensor.name if nc.partition_id_tensor else None
    )
    dbg_name = nc.dbg_addr.name if nc.dbg_addr is not None else None

    in_names, out_names, out_avals, zero_shapes = [], [], [], []
    for alloc in nc.m.functions[0].allocations:
        if not isinstance(alloc, mybir.MemoryLocationSet):
            continue
        name = alloc.memorylocations[0].name
        if alloc.kind == "ExternalInput":
            if name != partition_name:
                in_names.append(name)
        elif alloc.kind == "ExternalOutput":
            npdt = mybir.dt.np(alloc.dtype)
            out_avals.append(
                jax.core.ShapedArray(tuple(alloc.tensor_shape), npdt)
            )
            out_names.append(name)
            zero_shapes.append((tuple(alloc.tensor_shape), npdt))

    n_params = len(in_names)
    n_outs = len(out_names)
    all_in_names = in_names + out_names
    if partition_name is not None:
        all_in_names = all_in_names + [partition_name]
    donate = tuple(range(n_params, n_params + n_outs))

    def _body(*args):
        operands = list(args)
        if partition_name is not None:
            operands.append(bass2jax.partition_id_tensor())
        outs = bass2jax._bass_exec_p.bind(
            *operands,
            out_avals=tuple(out_avals),
            in_names=tuple(all_in_names),
            out_names=tuple(out_names),
            lowering_input_output_aliases=(),
            sim_require_finite=True,
            sim_require_nnan=True,
            nc=nc,
        )
        return tuple(outs)

    devices = jax.devices()[:NCORES]
    mesh = Mesh(np.asarray(devices), ("core",))
    in_specs = (PartitionSpec("core"),) * (n_params + n_outs)
    out_specs = (PartitionSpec("core"),) * n_outs
    sharded = jax.jit(
        shard_map(
            _body, mesh=mesh, in_specs=in_specs, out_specs=out_specs,
            check_rep=False,
        ),
        donate_argnums=donate,
        keep_unused=True,
    )

    def run(in_maps):
        def get(m, name):
            if name == dbg_name and name not in m:
                return np.zeros((1, 2), np.uint32)
            return np.asarray(m[name])

        concat_in = [
            np.concatenate([get(m, name) for m in in_maps], axis=0)
            for name in in_names
        ]
        concat_zeros = [
            np.zeros((NCORES * shp[0], *shp[1:]), dt) for shp, dt in zero_shapes
        ]
        out_arrs = sharded(*concat_in, *concat_zeros)
        return [
            {
                name: np.asarray(out_arrs[i]).reshape(
                    NCORES, *out_avals[i].shape
                )[c]
                for i, name in enumerate(out_names)
            }
            for c in range(NCORES)
        ]

    _RUNNER = run
    return run


class _Res:
    def __init__(self, results):
        self.results = results


def _run(in_maps, trace=False):
    return _Res(_get_runner()(in_maps))


def kernel(
    query,
    key,
    value,
    in_proj_weight,
    in_proj_bias,
    out_proj_weight,
    out_proj_bias,
):
    in_maps = _prep_in_maps(query, in_proj_weight, in_proj_bias, out_proj_weight)
    res = _run(in_maps, trace=False)
    return _gather(res.results, out_proj_bias)
